# revision 62
# baseline (speedup 1.0000x reference)
"""Tropical-max (max-plus) 2D convolution for Trainium2 (Bass/Tile).

Problem: out[b,o,y,x] = max_{c,ky,kx} ( imgs[b,c,y+ky-2,x+kx-2] + K[o,c,4-ky,4-kx] )
with imgs (4,16,64,64) f32, K (16,16,5,5) f32, stride=1, padding=2 (-inf padded).
The reference's patch extraction contaminates windows that touch the padding,
so the 2-pixel output border is -inf; reproduced on the host after gather.

Sharding: 8 cores, each computes 2 output channels (o = 2k, 2k+1) over the full
batch (tensor-parallel over C_out; weights passed per-core).

Per-core layout:
  SBUF partition p = c*8 + b*2 + yh   (c in 0..15, b in 0..3, yh in 0..1)
  Each partition holds a -inf padded fp16 image slab (36 rows x 68 cols,
  rows yh*32-2 .. yh*32+33, cols -2..65), staged on the host. X1 is the same
  slab shifted one element right so odd-kx window reads stay 4B-aligned
  (keeps the DVE packed perf modes engaged).
  Each of the 25 (ky,kx) taps produces one fp16 candidate plane [128, 4096]
  shared by both output channels (free dim = (o, y, x); each o-half gets its
  own per-partition weight add):
      tmp_t[:, o-half] = slab_window(ky,kx) + w[o, c(p), t]
  The adds are split between ScalarE activation-bias (15 taps @1x) and
  VectorE tensor_scalar (10 taps @4x). VectorE max-combines planes in-place
  (@2x fp16): its own planes chain into the accumulator first (filling its
  early idle time), then each completed ScalarE pair is folded in (pair max +
  chain merge). A 4-level partition-halving max tree then reduces over c
  (16 -> 1) in quarter-FD chunks so each realign SBUF->SBUF DMA (the compiler
  requires equal base partitions for 2-SBUF-input DVE ops) hides behind the
  other chunks' maxes; the final f32 level streams straight to the output DMA.
"""

import sys

for _p in ("/opt/trn_rl_repo",):
    if _p not in sys.path:
        sys.path.insert(0, _p)

import numpy as np

import concourse.bacc as bacc
import concourse.bass as bass
import concourse.mybir as mybir
from concourse import bass_utils
from concourse.tile import TileContext

B, C, H, W = 4, 16, 64, 64
O, KH, KW = 16, 5, 5
PAD = 2
N_CORES = 8
O_PER = O // N_CORES          # 2 output channels per core
YH = 2                        # y halves per image
HY = H // YH                  # 32 rows per half
ROWS = HY + 2 * PAD           # 36 slab rows
WPAD = W + 2 * PAD            # 68 slab cols
FREE = ROWS * WPAD            # 2448 slab elements / partition
ACC_F = HY * W                # 2048 acc elements / partition
NTERMS = KH * KW              # 25 taps
# Taps whose add runs on VectorE tensor_scalar @4x; rest on ScalarE.
DVE_TERMS = (0, 2, 4, 10, 12, 14, 20, 22, 24, 6)

f32 = mybir.dt.float32
f16 = mybir.dt.float16
Alu = mybir.AluOpType
AF = mybir.ActivationFunctionType
NEG_INF = float("-inf")


def _build_bass() -> bass.Bass:
    nc = bacc.Bacc(trn_type="TRN2")
    xs = nc.dram_tensor("xs", [128, FREE + 1], f16, kind="ExternalInput")
    wt = nc.dram_tensor("wt", [128, O_PER * NTERMS], f32, kind="ExternalInput")
    out = nc.dram_tensor("out", [O_PER, 8, ACC_F], f16, kind="ExternalOutput")

    with TileContext(nc) as tc:
        with (
            tc.tile_pool(name="const", bufs=1) as cpool,
            tc.tile_pool(name="tmp", bufs=12) as tpool,
        ):
            X0 = cpool.tile([128, FREE], f16, name="x0")
            X1 = cpool.tile([128, FREE], f16, name="x1")
            WSB = cpool.tile([128, O_PER * NTERMS], f32, name="wsb")

            # Issue order matters: the SP sequencer issues DMAs serially and
            # the first VectorE op waits on x0 AND wsb.
            nc.sync.dma_start(X0[:, :], xs[:, 1:FREE + 1])
            nc.sync.dma_start(WSB[:, :], wt[:, :])
            nc.sync.dma_start(X1[:, :], xs[:, 0:FREE])

            X0v = X0.rearrange("p (r j) -> p r j", j=WPAD)
            X1v = X1.rearrange("p (r j) -> p r j", j=WPAD)

            # Both output channels share one [128, 2*ACC_F] plane per tap
            # (each half gets its own per-partition weight add); the maxes,
            # the c-tree and the output DMA then run once at double FD,
            # halving instruction count and per-op overhead.
            F2 = O_PER * ACC_F

            def make_plane(t, on_dve):
                ky, kx = divmod(t, KW)
                if kx % 2 == 0:
                    win = X0v[:, ky:ky + HY, kx:kx + W]
                else:
                    win = X1v[:, ky:ky + HY, kx + 1:kx + 1 + W]
                tmp = tpool.tile([128, F2], f16, tag="tmp")
                for o in range(O_PER):
                    w_ap = WSB[:, o * NTERMS + t:o * NTERMS + t + 1]
                    halfv = tmp[:, o * ACC_F:(o + 1) * ACC_F].rearrange(
                        "p (r j) -> p r j", j=W
                    )
                    if on_dve:
                        nc.vector.tensor_scalar(halfv, win, w_ap, None, Alu.add)
                    else:
                        nc.scalar.activation(halfv, win, AF.Identity, bias=w_ap)
                return tmp

            dve_terms = [t for t in range(NTERMS) if t in DVE_TERMS]
            act_terms = [t for t in range(NTERMS) if t not in DVE_TERMS]

            # VectorE's own planes first: they merge into the running
            # accumulator while ScalarE is still producing, so VectorE's
            # early idle time is filled.
            acc = make_plane(dve_terms[0], True)
            for t in dve_terms[1:]:
                p = make_plane(t, True)
                nc.vector.tensor_tensor(acc[:, :], acc[:, :], p[:, :], Alu.max)

            # ScalarE planes are consumed pairwise the moment they complete:
            # one pair max (in-place) + one chain merge into acc.
            for i in range(0, len(act_terms) - 1, 2):
                a = make_plane(act_terms[i], False)
                b = make_plane(act_terms[i + 1], False)
                nc.vector.tensor_tensor(a[:, :], a[:, :], b[:, :], Alu.max)
                nc.vector.tensor_tensor(acc[:, :], acc[:, :], a[:, :], Alu.max)
            if len(act_terms) % 2:
                a = make_plane(act_terms[-1], False)
                for qh in range(4):
                    qs = slice(qh * F2 // 4, (qh + 1) * F2 // 4)
                    nc.vector.tensor_tensor(
                        acc[:, qs], acc[:, qs], a[:, qs], Alu.max
                    )

            # Partition-halving max tree over c (16 -> 1), quarter-chunked in
            # FD so each realign DMA hides behind the other chunks' maxes;
            # the final f32 level streams its output DMA per chunk.
            NQ = 4
            FQ = F2 // NQ
            out_r = out.rearrange("o p f -> p o f")
            cur = acc
            npart = 128
            for lvl in range(4):
                half = npart // 2
                nxt = cpool.tile([half, F2], f16, name=f"lv_{lvl}")
                for h in range(NQ):
                    sl = slice(h * FQ, (h + 1) * FQ)
                    hi = cpool.tile([half, FQ], f16, name=f"hi_{lvl}_{h}")
                    nc.sync.dma_start(hi[:, :], cur[half:npart, sl])
                    nc.vector.tensor_tensor(
                        nxt[:, sl], cur[0:half, sl], hi[:, :], Alu.max
                    )
                    if lvl == 3:
                        # Stream each finished f32 chunk straight out.
                        for o_idx in range(O_PER):
                            lo = o_idx * ACC_F
                            s2 = slice(max(h * FQ, lo),
                                       min((h + 1) * FQ, lo + ACC_F))
                            if s2.start < s2.stop:
                                nc.sync.dma_start(
                                    out_r[:, o_idx, s2.start - lo:s2.stop - lo],
                                    nxt[:, s2],
                                )
                cur = nxt
                npart = half

    nc.finalize()
    return nc


def _prepare_imgs(imgs_np: np.ndarray) -> np.ndarray:
    """Host staging: [128, FREE+1] fp16, -inf padded slab per partition.

    xs[p, 1 + r*68 + j] = imgs[b, c, yh*32 - 2 + r, j - 2] (valid) else -inf,
    for p = c*8 + b*2 + yh.
    """
    img_cb = imgs_np.transpose(1, 0, 2, 3).reshape(C * B, H, W)  # (c,b) major
    xs = np.full((128, FREE + 1), NEG_INF, dtype=np.float32)
    view = xs[:, 1:].reshape(128, ROWS, WPAD)
    # even p: yh=0 -> slab rows 2..35 = image rows 0..33
    view[0::2, PAD:ROWS, PAD:PAD + W] = img_cb[:, 0:ROWS - PAD, :]
    # odd p: yh=1 -> slab rows 0..33 = image rows 30..63
    view[1::2, 0:ROWS - PAD, PAD:PAD + W] = img_cb[:, H - (ROWS - PAD):H, :]
    return np.ascontiguousarray(xs.astype(np.float16))


def _prepare_weights(kernel_np: np.ndarray) -> list[np.ndarray]:
    """Per-core [128, O_PER*25] f32 weight tables.

    wt[p, o'*25 + ky*5 + kx] = kernel[2k+o', p//8, 4-ky, 4-kx].
    """
    kf = kernel_np[:, :, ::-1, ::-1].astype(np.float32)  # (O, C, KH, KW)
    maps = []
    for k in range(N_CORES):
        wt = np.empty((128, O_PER * NTERMS), dtype=np.float32)
        for o in range(O_PER):
            tab = kf[k * O_PER + o].reshape(C, NTERMS)  # (16, 25)
            wt[:, o * NTERMS:(o + 1) * NTERMS] = np.repeat(tab, 8, axis=0)
        maps.append(np.ascontiguousarray(wt))
    return maps


_NC_CACHE: dict[str, bass.Bass] = {}


def kernel(imgs, kernel, stride, padding):
    assert int(stride) == 1 and int(padding) == 2
    imgs_np = np.asarray(imgs, dtype=np.float32)
    kern_np = np.asarray(kernel, dtype=np.float32)
    assert imgs_np.shape == (B, C, H, W) and kern_np.shape == (O, C, KH, KW)

    if "nc" not in _NC_CACHE:
        _NC_CACHE["nc"] = _build_bass()
    nc = _NC_CACHE["nc"]

    xs = _prepare_imgs(imgs_np)
    wts = _prepare_weights(kern_np)
    in_maps = [{"xs": xs, "wt": wts[k]} for k in range(N_CORES)]
    res = bass_utils.run_bass_kernel_spmd(nc, in_maps, core_ids=list(range(N_CORES)))

    full = np.empty((B, O, H, W), dtype=np.float32)
    for k in range(N_CORES):
        r = res.results[k]["out"].reshape(O_PER, B, YH, HY, W)
        for o in range(O_PER):
            full[:, k * O_PER + o] = r[o].reshape(B, H, W)
    # Reference quirk: the whole 2-pixel output border is -inf.
    full[:, :, :PAD, :] = NEG_INF
    full[:, :, -PAD:, :] = NEG_INF
    full[:, :, :, :PAD] = NEG_INF
    full[:, :, :, -PAD:] = NEG_INF
    return full


# revision 63
# speedup vs baseline: 1.0028x; 1.0028x over previous
"""Tropical-max (max-plus) 2D convolution for Trainium2 (Bass/Tile).

Problem: out[b,o,y,x] = max_{c,ky,kx} ( imgs[b,c,y+ky-2,x+kx-2] + K[o,c,4-ky,4-kx] )
with imgs (4,16,64,64) f32, K (16,16,5,5) f32, stride=1, padding=2 (-inf padded).
The reference's patch extraction contaminates windows that touch the padding,
so the 2-pixel output border is -inf; reproduced on the host after gather.

Sharding: 8 cores, each computes 2 output channels (o = 2k, 2k+1) over the full
batch (tensor-parallel over C_out; weights passed per-core).

Per-core layout:
  SBUF partition p = c*8 + b*2 + yh   (c in 0..15, b in 0..3, yh in 0..1)
  Each partition holds a -inf padded fp16 image slab (36 rows x 68 cols,
  rows yh*32-2 .. yh*32+33, cols -2..65), staged on the host. X1 is the same
  slab shifted one element right so odd-kx window reads stay 4B-aligned
  (keeps the DVE packed perf modes engaged).
  Each of the 25 (ky,kx) taps produces one fp16 candidate plane [128, 4096]
  shared by both output channels (free dim = (o, y, x); each o-half gets its
  own per-partition weight add):
      tmp_t[:, o-half] = slab_window(ky,kx) + w[o, c(p), t]
  The adds are split between ScalarE activation-bias (15 taps @1x) and
  VectorE tensor_scalar (10 taps @4x). VectorE max-combines planes in-place
  (@2x fp16): its own planes chain into the accumulator first (filling its
  early idle time), then each completed ScalarE pair is folded in (pair max +
  chain merge). A 4-level partition-halving max tree then reduces over c
  (16 -> 1) in quarter-FD chunks so each realign SBUF->SBUF DMA (the compiler
  requires equal base partitions for 2-SBUF-input DVE ops) hides behind the
  other chunks' maxes; the final f32 level streams straight to the output DMA.
"""

import sys

for _p in ("/opt/trn_rl_repo",):
    if _p not in sys.path:
        sys.path.insert(0, _p)

import numpy as np

import concourse.bacc as bacc
import concourse.bass as bass
import concourse.mybir as mybir
from concourse import bass_utils
from concourse.tile import TileContext

B, C, H, W = 4, 16, 64, 64
O, KH, KW = 16, 5, 5
PAD = 2
N_CORES = 8
O_PER = O // N_CORES          # 2 output channels per core
YH = 2                        # y halves per image
HY = H // YH                  # 32 rows per half
ROWS = HY + 2 * PAD           # 36 slab rows
WPAD = W + 2 * PAD            # 68 slab cols
FREE = ROWS * WPAD            # 2448 slab elements / partition
ACC_F = HY * W                # 2048 acc elements / partition
NTERMS = KH * KW              # 25 taps
# Taps whose add runs on VectorE tensor_scalar @4x; rest on ScalarE.
DVE_TERMS = (0, 2, 4, 10, 12, 14, 20, 22, 24, 6)

f32 = mybir.dt.float32
f16 = mybir.dt.float16
Alu = mybir.AluOpType
AF = mybir.ActivationFunctionType
NEG_INF = float("-inf")


def _build_bass() -> bass.Bass:
    nc = bacc.Bacc(trn_type="TRN2")
    xs = nc.dram_tensor("xs", [128, FREE + 1], f16, kind="ExternalInput")
    wt = nc.dram_tensor("wt", [128, O_PER * NTERMS], f32, kind="ExternalInput")
    out = nc.dram_tensor("out", [O_PER, 8, ACC_F], f16, kind="ExternalOutput")

    with TileContext(nc) as tc:
        with (
            tc.tile_pool(name="const", bufs=1) as cpool,
            tc.tile_pool(name="tmp", bufs=12) as tpool,
        ):
            X0 = cpool.tile([128, FREE], f16, name="x0")
            X1 = cpool.tile([128, FREE], f16, name="x1")
            WSB = cpool.tile([128, O_PER * NTERMS], f32, name="wsb")

            # Issue order matters: the SP sequencer issues DMAs serially and
            # the first VectorE op waits on x0 AND wsb.
            # X0 split: chunk A covers every ky=0 window (slab rows 0..31),
            # so the first VectorE tap gates on the smaller transfer.
            CA = (HY - 1) * WPAD + W + KW - 1   # 2175: last col read by ky=0
            nc.sync.dma_start(X0[:, 0:CA], xs[:, 1:CA + 1])
            nc.sync.dma_start(WSB[:, :], wt[:, :])
            nc.sync.dma_start(X0[:, CA:FREE], xs[:, CA + 1:FREE + 1])
            nc.sync.dma_start(X1[:, :], xs[:, 0:FREE])

            X0v = X0.rearrange("p (r j) -> p r j", j=WPAD)
            X1v = X1.rearrange("p (r j) -> p r j", j=WPAD)

            # Both output channels share one [128, 2*ACC_F] plane per tap
            # (each half gets its own per-partition weight add); the maxes,
            # the c-tree and the output DMA then run once at double FD,
            # halving instruction count and per-op overhead.
            F2 = O_PER * ACC_F

            def make_plane(t, on_dve):
                ky, kx = divmod(t, KW)
                if kx % 2 == 0:
                    win = X0v[:, ky:ky + HY, kx:kx + W]
                else:
                    win = X1v[:, ky:ky + HY, kx + 1:kx + 1 + W]
                tmp = tpool.tile([128, F2], f16, tag="tmp")
                for o in range(O_PER):
                    w_ap = WSB[:, o * NTERMS + t:o * NTERMS + t + 1]
                    halfv = tmp[:, o * ACC_F:(o + 1) * ACC_F].rearrange(
                        "p (r j) -> p r j", j=W
                    )
                    if on_dve:
                        nc.vector.tensor_scalar(halfv, win, w_ap, None, Alu.add)
                    else:
                        nc.scalar.activation(halfv, win, AF.Identity, bias=w_ap)
                return tmp

            dve_terms = [t for t in range(NTERMS) if t in DVE_TERMS]
            act_terms = [t for t in range(NTERMS) if t not in DVE_TERMS]

            # VectorE's own planes first: they merge into the running
            # accumulator while ScalarE is still producing, so VectorE's
            # early idle time is filled.
            acc = make_plane(dve_terms[0], True)
            for t in dve_terms[1:]:
                p = make_plane(t, True)
                nc.vector.tensor_tensor(acc[:, :], acc[:, :], p[:, :], Alu.max)

            # ScalarE planes are consumed pairwise the moment they complete:
            # one pair max (in-place) + one chain merge into acc.
            for i in range(0, len(act_terms) - 1, 2):
                a = make_plane(act_terms[i], False)
                b = make_plane(act_terms[i + 1], False)
                nc.vector.tensor_tensor(a[:, :], a[:, :], b[:, :], Alu.max)
                nc.vector.tensor_tensor(acc[:, :], acc[:, :], a[:, :], Alu.max)
            if len(act_terms) % 2:
                a = make_plane(act_terms[-1], False)
                for qh in range(4):
                    qs = slice(qh * F2 // 4, (qh + 1) * F2 // 4)
                    nc.vector.tensor_tensor(
                        acc[:, qs], acc[:, qs], a[:, qs], Alu.max
                    )

            # Partition-halving max tree over c (16 -> 1), quarter-chunked in
            # FD so each realign DMA hides behind the other chunks' maxes;
            # the final f32 level streams its output DMA per chunk.
            NQ = 4
            FQ = F2 // NQ
            out_r = out.rearrange("o p f -> p o f")
            cur = acc
            npart = 128
            for lvl in range(4):
                half = npart // 2
                nxt = cpool.tile([half, F2], f16, name=f"lv_{lvl}")
                for h in range(NQ):
                    sl = slice(h * FQ, (h + 1) * FQ)
                    hi = cpool.tile([half, FQ], f16, name=f"hi_{lvl}_{h}")
                    nc.sync.dma_start(hi[:, :], cur[half:npart, sl])
                    nc.vector.tensor_tensor(
                        nxt[:, sl], cur[0:half, sl], hi[:, :], Alu.max
                    )
                    if lvl == 3:
                        # Stream each finished f32 chunk straight out.
                        for o_idx in range(O_PER):
                            lo = o_idx * ACC_F
                            s2 = slice(max(h * FQ, lo),
                                       min((h + 1) * FQ, lo + ACC_F))
                            if s2.start < s2.stop:
                                nc.sync.dma_start(
                                    out_r[:, o_idx, s2.start - lo:s2.stop - lo],
                                    nxt[:, s2],
                                )
                cur = nxt
                npart = half

    nc.finalize()
    return nc


def _prepare_imgs(imgs_np: np.ndarray) -> np.ndarray:
    """Host staging: [128, FREE+1] fp16, -inf padded slab per partition.

    xs[p, 1 + r*68 + j] = imgs[b, c, yh*32 - 2 + r, j - 2] (valid) else -inf,
    for p = c*8 + b*2 + yh.
    """
    img_cb = imgs_np.transpose(1, 0, 2, 3).reshape(C * B, H, W)  # (c,b) major
    xs = np.full((128, FREE + 1), NEG_INF, dtype=np.float32)
    view = xs[:, 1:].reshape(128, ROWS, WPAD)
    # even p: yh=0 -> slab rows 2..35 = image rows 0..33
    view[0::2, PAD:ROWS, PAD:PAD + W] = img_cb[:, 0:ROWS - PAD, :]
    # odd p: yh=1 -> slab rows 0..33 = image rows 30..63
    view[1::2, 0:ROWS - PAD, PAD:PAD + W] = img_cb[:, H - (ROWS - PAD):H, :]
    return np.ascontiguousarray(xs.astype(np.float16))


def _prepare_weights(kernel_np: np.ndarray) -> list[np.ndarray]:
    """Per-core [128, O_PER*25] f32 weight tables.

    wt[p, o'*25 + ky*5 + kx] = kernel[2k+o', p//8, 4-ky, 4-kx].
    """
    kf = kernel_np[:, :, ::-1, ::-1].astype(np.float32)  # (O, C, KH, KW)
    maps = []
    for k in range(N_CORES):
        wt = np.empty((128, O_PER * NTERMS), dtype=np.float32)
        for o in range(O_PER):
            tab = kf[k * O_PER + o].reshape(C, NTERMS)  # (16, 25)
            wt[:, o * NTERMS:(o + 1) * NTERMS] = np.repeat(tab, 8, axis=0)
        maps.append(np.ascontiguousarray(wt))
    return maps


_NC_CACHE: dict[str, bass.Bass] = {}


def kernel(imgs, kernel, stride, padding):
    assert int(stride) == 1 and int(padding) == 2
    imgs_np = np.asarray(imgs, dtype=np.float32)
    kern_np = np.asarray(kernel, dtype=np.float32)
    assert imgs_np.shape == (B, C, H, W) and kern_np.shape == (O, C, KH, KW)

    if "nc" not in _NC_CACHE:
        _NC_CACHE["nc"] = _build_bass()
    nc = _NC_CACHE["nc"]

    xs = _prepare_imgs(imgs_np)
    wts = _prepare_weights(kern_np)
    in_maps = [{"xs": xs, "wt": wts[k]} for k in range(N_CORES)]
    res = bass_utils.run_bass_kernel_spmd(nc, in_maps, core_ids=list(range(N_CORES)))

    full = np.empty((B, O, H, W), dtype=np.float32)
    for k in range(N_CORES):
        r = res.results[k]["out"].reshape(O_PER, B, YH, HY, W)
        for o in range(O_PER):
            full[:, k * O_PER + o] = r[o].reshape(B, H, W)
    # Reference quirk: the whole 2-pixel output border is -inf.
    full[:, :, :PAD, :] = NEG_INF
    full[:, :, -PAD:, :] = NEG_INF
    full[:, :, :, :PAD] = NEG_INF
    full[:, :, :, -PAD:] = NEG_INF
    return full
